# revision 1
# baseline (speedup 1.0000x reference)
"""MoE FFN (8 experts, top-2) on 8 TRN2 NeuronCores, expert-parallel.

Strategy:
  - Host: router (fp64 logits -> softmax -> top-2 -> renormalized combine
    weights), gather each expert's assigned tokens, pad to a common
    capacity C (SPMD: one program, per-core inputs).
  - Core e: full SwiGLU FFN for expert e over its C tokens in fp32r
    (TF32-like, 11-bit mantissa, full PE rate), combine-weight scaling on
    device; outputs [C, 1024].
  - Host: scatter-add per-expert outputs back into [B, S, D].

Layouts (host-prepared, DMA-friendly):
  xT   [8, 128, C]      x[idx].T split along d into 8 k-tiles
  gw/uw[32, 128, 8, 128] gate/up ^T tiled: [h_tile][d_sub][k][h]
  dw   [32, 128, 1024]  down^T tiled:     [h_tile][h_sub][dout]
  cwT  [128, C/128]     combine weights, partition-major
"""
import sys, os
for p in ("/opt/trn_rl_repo", os.path.join(os.path.dirname(os.path.abspath(__file__)))):
    if p not in sys.path:
        sys.path.insert(0, p)
import numpy as np

D_MODEL = 1024
D_INNER = 4096
N_EXPERTS = 8
TOP_K = 2
H_TILES = D_INNER // 128  # 32
K_TILES = D_MODEL // 128  # 8
CHUNK = 768               # token chunk for gate/up phase (two 384 halves)
PASS_T = 384              # token span per down-proj psum pass (3 x 128)


def fp32r_round(a: np.ndarray) -> np.ndarray:
    """Round fp32 to fp32r (11-bit mantissa, RNE) — matches walrus fp32_to_fp32r."""
    b = np.ascontiguousarray(a, dtype=np.float32).view(np.uint32).astype(np.int64)
    low = b & 0xFFF
    add = np.where((low > 0x800) | ((low == 0x800) & (((b >> 12) & 1) == 1)), 0x1000, 0)
    out = ((b + add) & ~0xFFF).astype(np.int64).astype(np.uint32, casting="unsafe")
    return out.view(np.float32).reshape(a.shape)


def _build_nc(C: int, reps: int = 1):
    import concourse.bass as bass
    import concourse.mybir as mybir
    import concourse.tile as tile
    from concourse import bacc
    from contextlib import nullcontext

    f32 = mybir.dt.float32
    f32r = mybir.dt.float32r
    Silu = mybir.ActivationFunctionType.Silu

    assert C % 128 == 0
    chunks = [CHUNK] * (C // CHUNK)
    if C % CHUNK:
        chunks.append(C % CHUNK)
    # chunk -> (gate/up moving-dim groups, down-proj pass sizes)
    def mm1_groups(tc_):
        return [tc_] if tc_ <= 512 else [tc_ - tc_ // 2, tc_ // 2]
    def mm2_passes(tc_):
        # (PT/128)*2 psum banks per pass; ps2 pool has 6 -> PT <= 384
        out = []
        while tc_ > 0:
            p = min(384, tc_)
            out.append(p)
            tc_ -= p
        return out

    nc = bacc.Bacc(None, target_bir_lowering=False)
    xT_d = nc.dram_tensor("xT", [K_TILES, 128, C], f32r, kind="ExternalInput")
    gw_d = nc.dram_tensor("gw", [H_TILES, 128, K_TILES, 128], f32r, kind="ExternalInput")
    uw_d = nc.dram_tensor("uw", [H_TILES, 128, K_TILES, 128], f32r, kind="ExternalInput")
    dw_d = nc.dram_tensor("dw", [H_TILES, 128, D_MODEL], f32r, kind="ExternalInput")
    cw_d = nc.dram_tensor("cwT", [128, C // 128], f32, kind="ExternalInput")
    y_d = nc.dram_tensor("y", [C, D_MODEL], f32, kind="ExternalOutput")

    TH = CHUNK // 2  # 384, moving dim for gate/up matmuls

    with tile.TileContext(nc) as tc:
        with (
            tc.tile_pool(name="xt", bufs=1) as xt_pool,
            tc.tile_pool(name="wgt", bufs=5) as wgt_pool,
            tc.tile_pool(name="dwp", bufs=8) as dw_pool,
            tc.tile_pool(name="hb", bufs=1) as hb_pool,
            tc.tile_pool(name="sg", bufs=2) as sg_pool,
            tc.tile_pool(name="yo", bufs=3) as y_pool,
            tc.tile_pool(name="cw", bufs=1) as cw_pool,
            tc.tile_pool(name="ps1", bufs=2, space="PSUM") as ps1,
            tc.tile_pool(name="ps2", bufs=6, space="PSUM") as ps2,
        ):
            cw_sb = cw_pool.tile([128, C // 128], f32)
            nc.sync.dma_start(cw_sb[:], cw_d[:])

            rep_ctx = tc.For_i(0, reps, 1) if reps > 1 else nullcontext()
            with rep_ctx:
              t0 = 0
              for TC in chunks:
                xt = xt_pool.tile([128, K_TILES, TC], f32r, tag="xt")
                # DRAM [k, d, t-slice] -> SBUF [d, k, t] (transpose on the DRAM side
                # so the SBUF AP stays partition-major)
                nc.sync.dma_start(
                    xt[:], xT_d[:, :, t0:t0 + TC].transpose([1, 0, 2])
                )
                hbuf = hb_pool.tile([128, H_TILES, TC], f32r, tag="hbuf")

                # ---- down-projection helpers: sweep dout halves; each half
                # streams only its 512-wide dw slice (halves dw traffic)
                n_sub = TC // 128
                assert n_sub <= 6  # ps2 banks
                def mm2_h(half, hi, yp):
                    ds_ = slice(half * 512, (half + 1) * 512)
                    dwt = dw_pool.tile([128, 512], f32r, tag="dw", name="dwt")
                    nc.sync.dma_start(dwt[:], dw_d[hi][:, ds_])
                    for s in range(n_sub):
                        ts_ = slice(s * 128, (s + 1) * 128)
                        if hi == 0:
                            yp[s] = ps2.tile([128, 512], f32, tag="yp", name="yp")
                        nc.tensor.matmul(
                            yp[s][:], hbuf[:, hi, ts_], dwt[:],
                            start=(hi == 0), stop=(hi == H_TILES - 1))
                def mm2_evac(half, yp):
                    ds_ = slice(half * 512, (half + 1) * 512)
                    for s in range(n_sub):
                        gcol = t0 // 128 + s
                        yt = y_pool.tile([128, 512], f32, tag="yt", name="yt")
                        nc.vector.tensor_scalar_mul(
                            yt[:], yp[s][:], cw_sb[:, gcol:gcol + 1])
                        nc.sync.dma_start(
                            y_d[t0 + s * 128: t0 + (s + 1) * 128, ds_], yt[:])

                # ---- gate/up + SwiGLU, one 128-row tile of d_inner at a time
                groups = mm1_groups(TC)
                for hi in range(H_TILES):
                    gw = wgt_pool.tile([128, K_TILES, 128], f32r, tag="gw")
                    nc.sync.dma_start(gw[:], gw_d[hi])
                    uw = wgt_pool.tile([128, K_TILES, 128], f32r, tag="uw")
                    nc.sync.dma_start(uw[:], uw_d[hi])
                    g0 = 0
                    for gsz in groups:
                        hs = slice(g0, g0 + gsz)
                        pg = ps1.tile([128, gsz], f32, tag="p1", name="pg", padded_shape=[128, 512])
                        for k in range(K_TILES):
                            nc.tensor.matmul(pg[:], gw[:, k, :], xt[:, k, hs],
                                             start=(k == 0), stop=(k == K_TILES - 1))
                        pu = ps1.tile([128, gsz], f32, tag="p1", name="pu", padded_shape=[128, 512])
                        for k in range(K_TILES):
                            nc.tensor.matmul(pu[:], uw[:, k, :], xt[:, k, hs],
                                             start=(k == 0), stop=(k == K_TILES - 1))
                        sg = sg_pool.tile([128, gsz], f32, tag="sg", name="sg", padded_shape=[128, 512])
                        nc.scalar.activation(sg[:], pg[:], Silu)
                        nc.vector.tensor_mul(hbuf[:, hi, hs], sg[:], pu[:])
                        g0 += gsz
                for half in range(2):
                    yp = [None] * n_sub
                    for hi in range(H_TILES):
                        mm2_h(half, hi, yp)
                    mm2_evac(half, yp)
                t0 += TC
    nc.finalize()
    return nc


_NC_CACHE: dict = {}


def _get_nc(C: int):
    if C not in _NC_CACHE:
        _NC_CACHE[C] = _build_nc(C)
    return _NC_CACHE[C]


def _route(x2d: np.ndarray, router_w: np.ndarray, router_b: np.ndarray):
    """fp64 router: returns (idx_per_expert, cw_per_expert) lists."""
    logits = x2d.astype(np.float64) @ router_w.astype(np.float64).T + router_b.astype(np.float64)
    m = logits.max(axis=-1, keepdims=True)
    p = np.exp(logits - m)
    p /= p.sum(axis=-1, keepdims=True)
    # top-2 (jax.lax.top_k picks largest; softmax is monotonic in logits)
    i1 = np.argmax(p, axis=-1)
    p_masked = p.copy()
    p_masked[np.arange(p.shape[0]), i1] = -1.0
    i2 = np.argmax(p_masked, axis=-1)
    p1 = p[np.arange(p.shape[0]), i1]
    p2 = p[np.arange(p.shape[0]), i2]
    denom = p1 + p2
    w1 = p1 / denom
    w2 = p2 / denom
    idxs, cws = [], []
    for e in range(N_EXPERTS):
        sel1 = np.nonzero(i1 == e)[0]
        sel2 = np.nonzero(i2 == e)[0]
        idx = np.concatenate([sel1, sel2])
        cw = np.concatenate([w1[sel1], w2[sel2]])
        idxs.append(idx)
        cws.append(cw.astype(np.float32))
    return idxs, cws


def _prep_core_inputs(x2d, idxs, cws, gate_w, up_w, down_w, C):
    in_maps = []
    for e in range(N_EXPERTS):
        idx = idxs[e]
        n = len(idx)
        xe = np.zeros((C, D_MODEL), np.float32)
        xe[:n] = x2d[idx]
        xT = fp32r_round(np.ascontiguousarray(xe.T)).reshape(K_TILES, 128, C)
        gw = fp32r_round(np.ascontiguousarray(
            gate_w[e].T.reshape(K_TILES, 128, H_TILES, 128).transpose(2, 1, 0, 3)))
        uw = fp32r_round(np.ascontiguousarray(
            up_w[e].T.reshape(K_TILES, 128, H_TILES, 128).transpose(2, 1, 0, 3)))
        dw = fp32r_round(np.ascontiguousarray(down_w[e].T).reshape(H_TILES, 128, D_MODEL))
        cw = np.zeros((C,), np.float32)
        cw[:n] = cws[e]
        cwT = np.ascontiguousarray(cw.reshape(-1, 128).T)
        in_maps.append({"xT": xT, "gw": gw, "uw": uw, "dw": dw, "cwT": cwT})
    return in_maps


def kernel(x, router_w, router_b, gate_w, up_w, down_w):
    from concourse.bass_utils import run_bass_kernel_spmd

    x = np.asarray(x, dtype=np.float32)
    router_w = np.asarray(router_w, dtype=np.float32)
    router_b = np.asarray(router_b, dtype=np.float32)
    gate_w = np.asarray(gate_w, dtype=np.float32)
    up_w = np.asarray(up_w, dtype=np.float32)
    down_w = np.asarray(down_w, dtype=np.float32)

    B, S, D = x.shape
    x2d = x.reshape(B * S, D)
    idxs, cws = _route(x2d, router_w, router_b)
    max_n = max(len(i) for i in idxs)
    C = max(256, ((max_n + 127) // 128) * 128)

    nc = _get_nc(C)
    in_maps = _prep_core_inputs(x2d, idxs, cws, gate_w, up_w, down_w, C)
    res = run_bass_kernel_spmd(nc, in_maps, core_ids=list(range(N_EXPERTS)), trace=False)

    out = np.zeros((B * S, D_MODEL), np.float32)
    for e in range(N_EXPERTS):
        n = len(idxs[e])
        np.add.at(out, idxs[e], res.results[e]["y"][:n])
    return out.reshape(B, S, D_MODEL)



# revision 2
# speedup vs baseline: 1.0359x; 1.0359x over previous
"""MoE FFN (8 experts, top-2) on 8 TRN2 NeuronCores, expert-parallel.

Strategy:
  - Host: router (fp64 logits -> softmax -> top-2 -> renormalized combine
    weights), gather each expert's assigned tokens, pad to a common
    capacity C (SPMD: one program, per-core inputs).
  - Core e: full SwiGLU FFN for expert e over its C tokens in bf16
    (full PE rate, fp32 PSUM accumulate), combine-weight scaling fused
    into the PSUM evacuation; outputs [C, 1024] fp32.
  - Host: scatter-add per-expert outputs back into [B, S, D].

Device kernel structure (single pass over all weights per invocation):
  - x kept resident in SBUF for the whole kernel: [128, 8, C] bf16.
  - d_inner split into 4 groups of 8 h-tiles (128 rows each). Per group:
      mm1: for each h-tile, gate/up chains over 8 k-tiles (moving dim =
           tokens, 512 wide), SiLU (scalar) * up (vector) -> hbuf bf16.
      mm2: for each (token-subtile, dout-half) output tile, one PSUM
           chain of 8 matmuls over the group's h-tiles; evacuation does
           y_acc = psum * cw (+ y_acc), last group DMAs y_acc out.
  - Weights stream exactly once per invocation (~25 MB bf16/core), far
    under the ~700us of tensor work -> fully hidden.

Layouts (host-prepared, DMA-friendly, all weights/x in bf16):
  xT   [128, 8, C]          x[idx].T tiled [d_sub][k][t]
  guw  [32, 128, 2, 8, 128] gate/up ^T tiled: [h_tile][d_sub][g|u][k][h]
  dw   [4, 128, 8, 2, 512]  down^T tiled: [h_group][h_sub][h_tile][d_half][dout]
  cwT  [128, C/128]         combine weights fp32, partition-major
"""
import sys, os
for p in ("/opt/trn_rl_repo", os.path.join(os.path.dirname(os.path.abspath(__file__)))):
    if p not in sys.path:
        sys.path.insert(0, p)
import numpy as np

D_MODEL = 1024
D_INNER = 4096
N_EXPERTS = 8
TOP_K = 2
H_TILES = D_INNER // 128  # 32
K_TILES = D_MODEL // 128  # 8
HG = 8                    # h-tiles per group (PSUM chain depth in mm2)
N_HG = H_TILES // HG      # 4


def _build_nc(C: int, reps: int = 1):
    import concourse.mybir as mybir
    import concourse.tile as tile
    from concourse import bacc
    from contextlib import nullcontext

    f32 = mybir.dt.float32
    bf16 = mybir.dt.bfloat16
    Silu = mybir.ActivationFunctionType.Silu
    Mult = mybir.AluOpType.mult
    Add = mybir.AluOpType.add

    assert C % 128 == 0
    NT = C // 128
    # moving-dim groups for the gate/up matmuls (tokens, <=512 per PSUM bank)
    mgroups = []
    t = 0
    while t < C:
        g = min(512, C - t)
        mgroups.append((t, g))
        t += g

    nc = bacc.Bacc(None, target_bir_lowering=False)
    xT_d = nc.dram_tensor("xT", [128, K_TILES, C], bf16, kind="ExternalInput")
    guw_d = nc.dram_tensor("guw", [H_TILES, 128, 2, K_TILES, 128], bf16, kind="ExternalInput")
    dw_d = nc.dram_tensor("dw", [N_HG, 128, HG, 2, 512], bf16, kind="ExternalInput")
    cw_d = nc.dram_tensor("cwT", [128, NT], f32, kind="ExternalInput")
    y_d = nc.dram_tensor("y", [C, D_MODEL], f32, kind="ExternalOutput")

    with tile.TileContext(nc) as tc:
        with (
            tc.tile_pool(name="xt", bufs=1) as xt_pool,
            tc.tile_pool(name="wgt", bufs=3) as wgt_pool,
            tc.tile_pool(name="dwp", bufs=2) as dw_pool,
            tc.tile_pool(name="hb", bufs=1) as hb_pool,
            tc.tile_pool(name="sg", bufs=3) as sg_pool,
            tc.tile_pool(name="ya", bufs=1) as ya_pool,
            tc.tile_pool(name="cw", bufs=1) as cw_pool,
            tc.tile_pool(name="ps1", bufs=4, space="PSUM") as ps1,
            tc.tile_pool(name="ps2", bufs=3, space="PSUM") as ps2,
        ):
            cw_sb = cw_pool.tile([128, NT], f32)
            nc.sync.dma_start(cw_sb[:], cw_d[:])
            yacc = ya_pool.tile([128, NT, D_MODEL], f32)

            rep_ctx = tc.For_i(0, reps, 1) if reps > 1 else nullcontext()
            with rep_ctx:
                xt = xt_pool.tile([128, K_TILES, C], bf16, tag="xt")
                nc.sync.dma_start(xt[:], xT_d[:])
                for hg in range(N_HG):
                    hbuf = hb_pool.tile([128, HG, C], bf16, tag="hb")
                    dwt = dw_pool.tile([128, HG, 2, 512], bf16, tag="dw")
                    nc.sync.dma_start(dwt[:], dw_d[hg])
                    # ---- mm1: gate/up + SwiGLU for this group's 8 h-tiles
                    for i in range(HG):
                        hi = hg * HG + i
                        guw = wgt_pool.tile([128, 2, K_TILES, 128], bf16, tag="w")
                        nc.sync.dma_start(guw[:], guw_d[hi])
                        for (t0, gsz) in mgroups:
                            hs = slice(t0, t0 + gsz)
                            pg = ps1.tile([128, gsz], f32, tag="p1", name="pg", padded_shape=[128, 512])
                            for k in range(K_TILES):
                                nc.tensor.matmul(pg[:], guw[:, 0, k, :], xt[:, k, hs],
                                                 start=(k == 0), stop=(k == K_TILES - 1))
                            pu = ps1.tile([128, gsz], f32, tag="p1", name="pu", padded_shape=[128, 512])
                            for k in range(K_TILES):
                                nc.tensor.matmul(pu[:], guw[:, 1, k, :], xt[:, k, hs],
                                                 start=(k == 0), stop=(k == K_TILES - 1))
                            sg = sg_pool.tile([128, gsz], bf16, tag="sg", name="sg", padded_shape=[128, 512])
                            nc.scalar.activation(sg[:], pg[:], Silu)
                            nc.vector.tensor_mul(hbuf[:, i, hs], sg[:], pu[:])
                    # ---- mm2: down-projection partial sums for this group
                    for ts in range(NT):
                        tsl = slice(ts * 128, (ts + 1) * 128)
                        for dh in range(2):
                            yp = ps2.tile([128, 512], f32, tag="p2", name="yp")
                            for i in range(HG):
                                nc.tensor.matmul(yp[:], hbuf[:, i, tsl], dwt[:, i, dh, :],
                                                 start=(i == 0), stop=(i == HG - 1))
                            ysl = yacc[:, ts, dh * 512:(dh + 1) * 512]
                            cws = cw_sb[:, ts:ts + 1]
                            if hg == 0:
                                nc.vector.tensor_scalar_mul(ysl, yp[:], cws)
                            else:
                                nc.vector.scalar_tensor_tensor(ysl, yp[:], cws, ysl, Mult, Add)
                            if hg == N_HG - 1:
                                nc.sync.dma_start(y_d[tsl, dh * 512:(dh + 1) * 512], ysl)
    nc.finalize()
    return nc


_NC_CACHE: dict = {}


def _get_nc(C: int):
    if C not in _NC_CACHE:
        _NC_CACHE[C] = _build_nc(C)
    return _NC_CACHE[C]


def _route(x2d: np.ndarray, router_w: np.ndarray, router_b: np.ndarray):
    """fp64 router: returns (idx_per_expert, cw_per_expert) lists."""
    logits = x2d.astype(np.float64) @ router_w.astype(np.float64).T + router_b.astype(np.float64)
    m = logits.max(axis=-1, keepdims=True)
    p = np.exp(logits - m)
    p /= p.sum(axis=-1, keepdims=True)
    # top-2 (jax.lax.top_k picks largest; softmax is monotonic in logits)
    i1 = np.argmax(p, axis=-1)
    p_masked = p.copy()
    p_masked[np.arange(p.shape[0]), i1] = -1.0
    i2 = np.argmax(p_masked, axis=-1)
    p1 = p[np.arange(p.shape[0]), i1]
    p2 = p[np.arange(p.shape[0]), i2]
    denom = p1 + p2
    w1 = p1 / denom
    w2 = p2 / denom
    idxs, cws = [], []
    for e in range(N_EXPERTS):
        sel1 = np.nonzero(i1 == e)[0]
        sel2 = np.nonzero(i2 == e)[0]
        idx = np.concatenate([sel1, sel2])
        cw = np.concatenate([w1[sel1], w2[sel2]])
        idxs.append(idx)
        cws.append(cw.astype(np.float32))
    return idxs, cws


def _prep_core_inputs(x2d, idxs, cws, gate_w, up_w, down_w, C):
    import ml_dtypes
    bf16 = ml_dtypes.bfloat16
    in_maps = []
    for e in range(N_EXPERTS):
        idx = idxs[e]
        n = len(idx)
        xe = np.zeros((C, D_MODEL), np.float32)
        xe[:n] = x2d[idx]
        xT = np.ascontiguousarray(
            xe.T.reshape(K_TILES, 128, C).transpose(1, 0, 2)).astype(bf16)
        g_t = gate_w[e].T.reshape(K_TILES, 128, H_TILES, 128).transpose(2, 1, 0, 3)
        u_t = up_w[e].T.reshape(K_TILES, 128, H_TILES, 128).transpose(2, 1, 0, 3)
        guw = np.ascontiguousarray(np.stack([g_t, u_t], axis=2)).astype(bf16)
        dw = np.ascontiguousarray(
            down_w[e].T.reshape(N_HG, HG, 128, 2, 512).transpose(0, 2, 1, 3, 4)).astype(bf16)
        cw = np.zeros((C,), np.float32)
        cw[:n] = cws[e]
        cwT = np.ascontiguousarray(cw.reshape(-1, 128).T)
        in_maps.append({"xT": xT, "guw": guw, "dw": dw, "cwT": cwT})
    return in_maps


def kernel(x, router_w, router_b, gate_w, up_w, down_w):
    from concourse.bass_utils import run_bass_kernel_spmd

    x = np.asarray(x, dtype=np.float32)
    router_w = np.asarray(router_w, dtype=np.float32)
    router_b = np.asarray(router_b, dtype=np.float32)
    gate_w = np.asarray(gate_w, dtype=np.float32)
    up_w = np.asarray(up_w, dtype=np.float32)
    down_w = np.asarray(down_w, dtype=np.float32)

    B, S, D = x.shape
    x2d = x.reshape(B * S, D)
    idxs, cws = _route(x2d, router_w, router_b)
    max_n = max(len(i) for i in idxs)
    C = max(256, ((max_n + 127) // 128) * 128)

    nc = _get_nc(C)
    in_maps = _prep_core_inputs(x2d, idxs, cws, gate_w, up_w, down_w, C)
    res = run_bass_kernel_spmd(nc, in_maps, core_ids=list(range(N_EXPERTS)), trace=False)

    out = np.zeros((B * S, D_MODEL), np.float32)
    for e in range(N_EXPERTS):
        n = len(idxs[e])
        np.add.at(out, idxs[e], res.results[e]["y"][:n])
    return out.reshape(B, S, D_MODEL)


# revision 8
# speedup vs baseline: 1.1856x; 1.1446x over previous
"""MoE FFN (8 experts, top-2) on 8 TRN2 NeuronCores, expert-parallel.

Strategy:
  - Host: router (fp64 logits -> softmax -> top-2 -> renormalized combine
    weights), gather each expert's assigned tokens, pad to a common
    capacity C (SPMD: one program, per-core inputs).
  - Core e: full SwiGLU FFN for expert e over its C tokens in bf16
    (full PE rate, fp32 PSUM accumulate), combine-weight scaling fused
    into the PSUM evacuation; outputs [C, 1024] fp32.
  - Host: scatter-add per-expert outputs back into [B, S, D].

Device kernel structure (single pass over all weights per invocation):
  - x kept resident in SBUF for the whole kernel: [128, 8, C] bf16.
  - d_inner split into 4 groups of 8 h-tiles (128 rows each). Per group:
      mm1: for each h-tile, gate/up chains over 8 k-tiles (moving dim =
           tokens, 512 wide), SiLU (scalar) * up (vector) -> hbuf bf16.
      mm2: for each (token-subtile, dout-half) output tile, one PSUM
           chain of 8 matmuls over the group's h-tiles; evacuation does
           y_acc = psum * cw (+ y_acc), last group DMAs y_acc out.
  - Weights stream exactly once per invocation (~25 MB bf16/core), far
    under the ~700us of tensor work -> fully hidden.

Layouts (host-prepared, DMA-friendly, all weights/x in bf16):
  xT   [128, 8, C]          x[idx].T tiled [d_sub][k][t]
  guw  [32, 128, 2, 8, 128] gate/up ^T tiled: [h_tile][d_sub][g|u][k][h]
  dw   [4, 128, 8, 2, 512]  down^T tiled: [h_group][h_sub][h_tile][d_half][dout]
  cwT  [128, C/128]         combine weights fp32, partition-major
"""
import sys, os
for p in ("/opt/trn_rl_repo", os.path.join(os.path.dirname(os.path.abspath(__file__)))):
    if p not in sys.path:
        sys.path.insert(0, p)
import numpy as np

D_MODEL = 1024
D_INNER = 4096
N_EXPERTS = 8
TOP_K = 2
H_TILES = D_INNER // 128  # 32
K_TILES = D_MODEL // 128  # 8
HG = 8                    # h-tiles per group (PSUM chain depth in mm2)
N_HG = H_TILES // HG      # 4


def _build_nc(C: int, reps: int = 1):
    import concourse.mybir as mybir
    import concourse.tile as tile
    from concourse import bacc
    from contextlib import nullcontext

    f32 = mybir.dt.float32
    bf16 = mybir.dt.bfloat16
    Silu = mybir.ActivationFunctionType.Silu
    Mult = mybir.AluOpType.mult
    Add = mybir.AluOpType.add

    assert C % 32 == 0
    NT = (C + 127) // 128
    # moving-dim groups for the gate/up matmuls (tokens, <=512 per PSUM
    # bank). Equal-ish sizes: a tiny tail group would be LDWEIGHTS-bound.
    n_mg = (C + 511) // 512
    base = (C // n_mg) // 32 * 32
    sizes = [base] * n_mg
    rem = C - base * n_mg
    i = 0
    while rem > 0:
        sizes[i] += 32
        rem -= 32
        i = (i + 1) % n_mg
    mgroups = []
    t = 0
    for g in sizes:
        mgroups.append((t, g))
        t += g
    # split x into two SBUF tiles at a group boundary so compute can
    # start after the first half's DMA lands
    xsplit = mgroups[(n_mg + 1) // 2][0]

    nc = bacc.Bacc(None, target_bir_lowering=False)
    xT_d = nc.dram_tensor("xT", [128, K_TILES, C], bf16, kind="ExternalInput")
    guw_d = nc.dram_tensor("guw", [H_TILES, 128, 2, K_TILES, 128], bf16, kind="ExternalInput")
    dw_d = nc.dram_tensor("dw", [N_HG, 128, HG, 2, 512], bf16, kind="ExternalInput")
    cw_d = nc.dram_tensor("cwT", [128, NT], f32, kind="ExternalInput")
    y_d = nc.dram_tensor("y", [C, D_MODEL], f32, kind="ExternalOutput")

    with tile.TileContext(nc) as tc:
        with (
            tc.tile_pool(name="xt", bufs=1) as xt_pool,
            tc.tile_pool(name="wgt", bufs=3) as wgt_pool,
            tc.tile_pool(name="dwp", bufs=2) as dw_pool,
            tc.tile_pool(name="hb", bufs=1) as hb_pool,
            tc.tile_pool(name="sg", bufs=3) as sg_pool,
            tc.tile_pool(name="ya", bufs=1) as ya_pool,
            tc.tile_pool(name="cw", bufs=1) as cw_pool,
            tc.tile_pool(name="ps1", bufs=4, space="PSUM") as ps1,
            tc.tile_pool(name="ps2", bufs=3, space="PSUM") as ps2,
        ):
            cw_sb = cw_pool.tile([128, NT], f32)
            nc.sync.dma_start(cw_sb[:], cw_d[:])
            yacc = ya_pool.tile([128, NT, D_MODEL], f32)

            rep_ctx = tc.For_i(0, reps, 1) if reps > 1 else nullcontext()
            with rep_ctx:
                xta = xt_pool.tile([128, K_TILES, xsplit], bf16, tag="xta", name="xta")
                nc.sync.dma_start(xta[:], xT_d[:, :, 0:xsplit])
                xtb = xt_pool.tile([128, K_TILES, C - xsplit], bf16, tag="xtb", name="xtb")
                nc.sync.dma_start(xtb[:], xT_d[:, :, xsplit:C])

                def xslice(t0, gsz):
                    if t0 < xsplit:
                        assert t0 + gsz <= xsplit
                        return xta, slice(t0, t0 + gsz)
                    return xtb, slice(t0 - xsplit, t0 - xsplit + gsz)

                for hg in range(N_HG):
                    hbuf = hb_pool.tile([128, HG, C], bf16, tag="hb")
                    dwt = dw_pool.tile([128, HG, 2, 512], bf16, tag="dw")
                    nc.sync.dma_start(dwt[:], dw_d[hg])
                    # ---- mm1: gate/up + SwiGLU for this group's 8 h-tiles
                    for i in range(HG):
                        hi = hg * HG + i
                        guw = wgt_pool.tile([128, 2, K_TILES, 128], bf16, tag="w")
                        nc.sync.dma_start(guw[:], guw_d[hi])
                        for (t0, gsz) in mgroups:
                            hs = slice(t0, t0 + gsz)
                            xtile, xs = xslice(t0, gsz)
                            pg = ps1.tile([128, gsz], f32, tag="p1", name="pg", padded_shape=[128, 512])
                            for k in range(K_TILES):
                                nc.tensor.matmul(pg[:], guw[:, 0, k, :], xtile[:, k, xs],
                                                 start=(k == 0), stop=(k == K_TILES - 1))
                            pu = ps1.tile([128, gsz], f32, tag="p1", name="pu", padded_shape=[128, 512])
                            for k in range(K_TILES):
                                nc.tensor.matmul(pu[:], guw[:, 1, k, :], xtile[:, k, xs],
                                                 start=(k == 0), stop=(k == K_TILES - 1))
                            sg = sg_pool.tile([128, gsz], bf16, tag="sg", name="sg", padded_shape=[128, 512])
                            nc.scalar.activation(sg[:], pg[:], Silu)
                            nc.vector.tensor_mul(hbuf[:, i, hs], sg[:], pu[:])
                    # ---- mm2: down-projection partial sums for this group
                    for ts in range(NT):
                        tw = min(128, C - ts * 128)
                        tsl = slice(ts * 128, ts * 128 + tw)
                        for dh in range(2):
                            yp = ps2.tile([128, 512], f32, tag="p2", name="yp")
                            for i in range(HG):
                                nc.tensor.matmul(yp[:tw, :], hbuf[:, i, tsl], dwt[:, i, dh, :],
                                                 start=(i == 0), stop=(i == HG - 1))
                            ysl = yacc[:tw, ts, dh * 512:(dh + 1) * 512]
                            cws = cw_sb[:tw, ts:ts + 1]
                            if hg == 0:
                                nc.vector.tensor_scalar_mul(ysl, yp[:tw, :], cws)
                            else:
                                nc.vector.scalar_tensor_tensor(ysl, yp[:tw, :], cws, ysl, Mult, Add)
                            if hg == N_HG - 1:
                                nc.sync.dma_start(y_d[tsl, dh * 512:(dh + 1) * 512], ysl)
    nc.finalize()
    return nc


_NC_CACHE: dict = {}


def _get_nc(C: int):
    if C not in _NC_CACHE:
        _NC_CACHE[C] = _build_nc(C)
    return _NC_CACHE[C]


def _route(x2d: np.ndarray, router_w: np.ndarray, router_b: np.ndarray):
    """fp64 router: returns (idx_per_expert, cw_per_expert) lists."""
    logits = x2d.astype(np.float64) @ router_w.astype(np.float64).T + router_b.astype(np.float64)
    m = logits.max(axis=-1, keepdims=True)
    p = np.exp(logits - m)
    p /= p.sum(axis=-1, keepdims=True)
    # top-2 (jax.lax.top_k picks largest; softmax is monotonic in logits)
    i1 = np.argmax(p, axis=-1)
    p_masked = p.copy()
    p_masked[np.arange(p.shape[0]), i1] = -1.0
    i2 = np.argmax(p_masked, axis=-1)
    p1 = p[np.arange(p.shape[0]), i1]
    p2 = p[np.arange(p.shape[0]), i2]
    denom = p1 + p2
    w1 = p1 / denom
    w2 = p2 / denom
    idxs, cws = [], []
    for e in range(N_EXPERTS):
        sel1 = np.nonzero(i1 == e)[0]
        sel2 = np.nonzero(i2 == e)[0]
        idx = np.concatenate([sel1, sel2])
        cw = np.concatenate([w1[sel1], w2[sel2]])
        idxs.append(idx)
        cws.append(cw.astype(np.float32))
    return idxs, cws


def _prep_core_inputs(x2d, idxs, cws, gate_w, up_w, down_w, C):
    import ml_dtypes
    bf16 = ml_dtypes.bfloat16
    in_maps = []
    for e in range(N_EXPERTS):
        idx = idxs[e]
        n = len(idx)
        xe = np.zeros((C, D_MODEL), np.float32)
        xe[:n] = x2d[idx]
        xT = np.ascontiguousarray(
            xe.T.reshape(K_TILES, 128, C).transpose(1, 0, 2)).astype(bf16)
        g_t = gate_w[e].T.reshape(K_TILES, 128, H_TILES, 128).transpose(2, 1, 0, 3)
        u_t = up_w[e].T.reshape(K_TILES, 128, H_TILES, 128).transpose(2, 1, 0, 3)
        guw = np.ascontiguousarray(np.stack([g_t, u_t], axis=2)).astype(bf16)
        dw = np.ascontiguousarray(
            down_w[e].T.reshape(N_HG, HG, 128, 2, 512).transpose(0, 2, 1, 3, 4)).astype(bf16)
        NT = (C + 127) // 128
        cw = np.zeros((NT * 128,), np.float32)
        cw[:n] = cws[e]
        cwT = np.ascontiguousarray(cw.reshape(-1, 128).T)
        in_maps.append({"xT": xT, "guw": guw, "dw": dw, "cwT": cwT})
    return in_maps


def kernel(x, router_w, router_b, gate_w, up_w, down_w):
    from concourse.bass_utils import run_bass_kernel_spmd

    x = np.asarray(x, dtype=np.float32)
    router_w = np.asarray(router_w, dtype=np.float32)
    router_b = np.asarray(router_b, dtype=np.float32)
    gate_w = np.asarray(gate_w, dtype=np.float32)
    up_w = np.asarray(up_w, dtype=np.float32)
    down_w = np.asarray(down_w, dtype=np.float32)

    B, S, D = x.shape
    x2d = x.reshape(B * S, D)
    idxs, cws = _route(x2d, router_w, router_b)
    max_n = max(len(i) for i in idxs)
    C = max(256, ((max_n + 31) // 32) * 32)

    nc = _get_nc(C)
    in_maps = _prep_core_inputs(x2d, idxs, cws, gate_w, up_w, down_w, C)
    res = run_bass_kernel_spmd(nc, in_maps, core_ids=list(range(N_EXPERTS)), trace=False)

    out = np.zeros((B * S, D_MODEL), np.float32)
    for e in range(N_EXPERTS):
        n = len(idxs[e])
        np.add.at(out, idxs[e], res.results[e]["y"][:n])
    return out.reshape(B, S, D_MODEL)
